# revision 27
# baseline (speedup 1.0000x reference)
"""Trainium2 Bass kernel for DendSeqNet (dendritic spiking net, T=64 steps).

v3 strategy (fp16 GEMM is forced: the net is chaotic — fp8 in any split
configuration and fp16 *state* both fail the 2e-2 gate; fp16 GEMM inputs
with f32 state are bit-exact vs the reference):
  - Pure data-parallel over batch: 8 cores x 16 batch elements.
  - fp16 GEMM, weights resident in SBUF, x streamed per 8-step chunk in a
    chunk-major layout (8KB contiguous runs -> full-rate DMA, single DMA
    per chunk half).
  - Startup: chunk 0+1 GEMMs interleaved per weight tile in DMA-arrival
    order, so the PE tracks the 47us weight stream with zero stall.
  - Recurrence engine split per step:
      DVE:  5x scalar_tensor_tensor (md update/reset, us update/reset, ish)
      ACT:  Sign(md-10) -> zd_s {-1,1}; Sign(us-10) -> zb_s (out-GEMM
            input; w2 is pre-halved and a +c0 rowsum bias is applied at the
            cot PSUM evacuation); PSUM->SBUF cur evacuations (fp16)
      Pool: zs2 = 0.5*(zd_s0+zd_s1)+1, out-dendrite LIF (fused STT forms)
    Emission interleaves rec steps of chunk ch-1 into the GEMM of chunk ch
    (1 step per 2 weight tiles) so the in-order ACT queue serves both
    streams without blocking PSUM evacuation.
  - Tail: last chunk emitted in 2-step column blocks with rec(6)/rec(7)
    steps front-loaded between blocks; only ~4 rec steps + out drain
    remain after the final matmul.
  - Host pre-filter: x~(t) = 0.8 x~(t-1) + x(t-1). Host post-filter: the
    readout is linear in the out-dendrite spikes zq, so the device emits
    zq [40, T, BS] and the host applies the [T,T] double-exponential
    kernel.
"""

import numpy as np
from contextlib import ExitStack

import concourse.bacc as bacc
import concourse.tile as tile
import concourse.mybir as mybir
from concourse.bass_utils import run_bass_kernel_spmd

F32 = mybir.dt.float32
F16 = mybir.dt.float16
I8 = mybir.dt.int8
OP = mybir.AluOpType
AF = mybir.ActivationFunctionType

N_CORES = 8
T, B, FS2 = 64, 128, 4096
HC, SPL1, H1 = 2, 2048, 2048
OC, SPL2, OUT = 4, 512, 10
BS = B // N_CORES          # 16 batch rows per core
ROWS = T * BS              # 1024 GEMM cols per core
KT = SPL1 // 128           # 16 contraction tiles per channel
MT = H1 // 128             # 16 output tiles per channel
GT = HC * KT               # 32 feature tiles of xT
NT = HC * MT               # 32 dendrite tiles
ST = H1 // 128             # 16 somatic tiles
CH = 8                     # steps per chunk
NCH = T // CH              # 8 chunks
NCOL = CH * BS             # 128 moving cols per chunk
TB = 2                     # tail block: steps per block in the last chunk
NTB = CH // TB             # tail blocks


def build_nc(repeat=1):
    nc = bacc.Bacc("TRN2", target_bir_lowering=False)

    xT = nc.dram_tensor("xT", [128, NCH, GT, NCOL], F16, kind="ExternalInput")
    wh = nc.dram_tensor("wh", [HC, MT, 128, KT, 128], F16,
                        kind="ExternalInput")
    w2 = nc.dram_tensor("w2", [ST, 128, 40], F16, kind="ExternalInput")
    outd = nc.dram_tensor("outd", [40, T, BS], I8, kind="ExternalOutput")

    dve = nc.vector
    act = nc.scalar
    gp = nc.gpsimd

    with tile.TileContext(nc) as tc:
      for _rep in range(repeat):
        with ExitStack() as ctx:
            persist = ctx.enter_context(tc.tile_pool(name="persist", bufs=1))
            xapool = ctx.enter_context(tc.tile_pool(name="xap", bufs=2))
            xbpool = ctx.enter_context(tc.tile_pool(name="xbp", bufs=2))
            curpool = ctx.enter_context(tc.tile_pool(name="curp", bufs=3))
            zbpool = ctx.enter_context(tc.tile_pool(name="zbp", bufs=2))
            zdpool = ctx.enter_context(tc.tile_pool(name="zdp", bufs=1))
            zspool = ctx.enter_context(tc.tile_pool(name="zsp", bufs=1))
            zqpool = ctx.enter_context(tc.tile_pool(name="zqp", bufs=2))
            copool = ctx.enter_context(tc.tile_pool(name="cop", bufs=1))
            gpsum = ctx.enter_context(
                tc.tile_pool(name="gpsum", bufs=4, space="PSUM"))
            tpsum = ctx.enter_context(
                tc.tile_pool(name="tpsum", bufs=2, space="PSUM"))
            opsum = ctx.enter_context(
                tc.tile_pool(name="opsum", bufs=1, space="PSUM"))
            dpsum = ctx.enter_context(
                tc.tile_pool(name="dpsum", bufs=1, space="PSUM"))

            # ---- persistent SBUF ----
            wht = [persist.tile([128, KT, 128], F16, tag=f"w{cm}",
                                name=f"wht{cm}")
                   for cm in range(HC * MT)]
            w2s = persist.tile([128, ST, 40], F16, tag="w2s")
            md = persist.tile([128, NT, BS], F32, tag="md")    # dend m=10v
            us = persist.tile([128, ST, BS], F32, tag="us")    # soma m=10v
            ish = persist.tile([128, ST, BS], F32, tag="ish")  # soma current
            qo = persist.tile([40, BS], F32, tag="qo")         # outd m=10v
            ido = persist.tile([40, BS], F32, tag="ido")       # outd current

            dve.memset(md[:], 0.0)
            dve.memset(us[:], 0.0)
            gp.memset(ish[:], 0.0)
            gp.memset(qo[:], 0.0)
            gp.memset(ido[:], 0.0)

            # ---- startup DMAs, consumption order, all on SP ----
            xa_tiles = {}
            xb_tiles = {}

            def fetch_x_half(ch, half):
                pool, tiles = ((xapool, xa_tiles) if half == 0
                               else (xbpool, xb_tiles))
                xf = pool.tile([128, KT, NCOL], F16, tag="xh",
                               name=f"x{ch}h{half}")
                s = slice(0, KT) if half == 0 else slice(KT, GT)
                nc.sync.dma_start(xf[:], xT[:, ch, s, :])
                tiles[ch] = xf

            def fetch_x(ch):
                fetch_x_half(ch, 0)
                fetch_x_half(ch, 1)

            # just-in-time startup order: the chunk0/1 GEMM consumes
            # (w_i, x-halves) at ~1.7us/tile; DMA delivers at ~1.5us/tile.
            xf0 = xapool.tile([128, KT, NCOL], F16, tag="xh", name="x0h0")
            xa_tiles[0] = xf0
            nc.sync.dma_start(xf0[:, 0:KT // 2, :], xT[:, 0, 0:KT // 2, :])
            nc.sync.dma_start(wht[0][:, 0:KT // 2, :],
                              wh[0, 0, :, 0:KT // 2, :])
            nc.sync.dma_start(xf0[:, KT // 2:KT, :], xT[:, 0, KT // 2:KT, :])
            nc.sync.dma_start(wht[0][:, KT // 2:KT, :],
                              wh[0, 0, :, KT // 2:KT, :])
            fetch_x_half(1, 0)
            for cm in range(1, 13):
                nc.sync.dma_start(wht[cm][:], wh[cm // MT, cm % MT])
            fetch_x_half(0, 1)
            fetch_x_half(1, 1)
            for cm in range(13, NT):
                nc.sync.dma_start(wht[cm][:], wh[cm // MT, cm % MT])
            nc.sync.dma_start(w2s[:], w2[:].rearrange("g p q -> p g q"))
            for ch in range(2, NCH):
                fetch_x(ch)

            cur_tiles = {}
            zb_tiles = {}
            co_tiles = {}

            def emit_mm_group(ch, cm, out_ps, cols=slice(0, NCOL)):
                """16 accumulating matmuls for weight tile cm into out_ps."""
                c, m = divmod(cm, MT)
                xf = xa_tiles[ch] if c == 0 else xb_tiles[ch]
                wt = wht[cm]
                for k in range(KT):
                    nc.tensor.matmul(
                        out_ps[:], wt[:, k, :], xf[:, k, cols],
                        start=(k == 0), stop=(k == KT - 1))

            def emit_bundle(ch, q):
                """4 weight tiles (4q..4q+3) -> one psum bank -> one evac."""
                ps = gpsum.tile([128, 4, NCOL], F32, tag="gps")
                for j in range(4):
                    emit_mm_group(ch, 4 * q + j, ps[:, j, :])
                act.copy(cur_tiles[ch][:, 4 * q:4 * q + 4, :], ps[:])

            def emit_rec_step(ch, tl, soma_pool=False):
                """one recurrence step. soma_pool=True moves the somatic
                ops to Pool as TS/TT pairs (bit-identical rounding to the
                DVE STT forms) so tail chunks run DVE and Pool in
                parallel at ~1.8us/step."""
                curt = cur_tiles[ch]
                zbt = zb_tiles[ch]
                col = slice(tl * BS, (tl + 1) * BS)
                if soma_pool:
                    # DVE: dendrite only
                    dve.scalar_tensor_tensor(
                        md[:], md[:], 0.9, curt[:, :, col], OP.mult, OP.add)
                    zdt = zdpool.tile([128, NT, BS], F16, tag="zd")
                    dve.tensor_scalar(zdt[:], md[:], 10.0, None, OP.is_gt)
                    dve.scalar_tensor_tensor(
                        md[:], md[:], 10.0, md[:], OP.is_le, OP.mult)
                    # Pool: zs2 + full soma
                    zs2 = zspool.tile([128, ST, BS], F16, tag="zs2")
                    gp.tensor_tensor(
                        zs2[:], zdt[:, 0:MT, :], zdt[:, MT:NT, :], OP.add)
                    gp.tensor_scalar(us[:], us[:], 0.9, None, OP.mult)
                    gp.tensor_tensor(us[:], us[:], ish[:], OP.add)
                    gp.tensor_scalar(
                        zbt[:, :, col], us[:], 10.0, None, OP.is_gt)
                    dve.scalar_tensor_tensor(
                        us[:], us[:], 10.0, us[:], OP.is_le, OP.mult)
                    gp.tensor_scalar(ish[:], ish[:], 0.8, None, OP.mult)
                    gp.tensor_tensor(ish[:], ish[:], zs2[:], OP.add)
                    return
                # D1: m = 0.9 m + cur(t)
                dve.scalar_tensor_tensor(
                    md[:], md[:], 0.9, curt[:, :, col], OP.mult, OP.add)
                # zd = (m > 10) in {0,1} (int8; exact at the threshold)
                zdt = zdpool.tile([128, NT, BS], F16, tag="zd")
                dve.tensor_scalar(zdt[:], md[:], 10.0, None, OP.is_gt)
                # D3: dendrite reset m = (m<=10)*m
                dve.scalar_tensor_tensor(
                    md[:], md[:], 10.0, md[:], OP.is_le, OP.mult)
                # S4: u = 0.9 u + i_s(old)
                dve.scalar_tensor_tensor(
                    us[:], us[:], 0.9, ish[:], OP.mult, OP.add)
                # Pool: zs2 = zd0 + zd1 in {0,1,2}
                zs2 = zspool.tile([128, ST, BS], F16, tag="zs2")
                gp.tensor_tensor(
                    zs2[:], zdt[:, 0:MT, :], zdt[:, MT:NT, :], OP.add)
                # z_s = (u > 10) -> fp16 for the out GEMM
                dve.tensor_scalar(zbt[:, :, col], us[:], 10.0, None, OP.is_gt)
                # S6: soma reset u = (u<=10)*u
                dve.scalar_tensor_tensor(
                    us[:], us[:], 10.0, us[:], OP.is_le, OP.mult)
                # i_s = 0.8 i_s + zsum
                dve.scalar_tensor_tensor(
                    ish[:], ish[:], 0.8, zs2[:], OP.mult, OP.add)

            def new_chunk_tiles(ch):
                cur_tiles[ch] = curpool.tile([128, NT, NCOL], F32, tag="cur",
                                             name=f"cur{ch}")
                zb_tiles[ch] = zbpool.tile([128, ST, NCOL], F16, tag="zb",
                                           name=f"zb{ch}")

            def emit_gemm01():
                """chunks 0+1 interleaved per weight tile (DMA order)."""
                new_chunk_tiles(0)
                new_chunk_tiles(1)
                for q in range(NT // 4):
                    psa = gpsum.tile([128, 4, NCOL], F32, tag="gps")
                    psb = gpsum.tile([128, 4, NCOL], F32, tag="gps")
                    for j in range(4):
                        emit_mm_group(0, 4 * q + j, psa[:, j, :])
                        emit_mm_group(1, 4 * q + j, psb[:, j, :])
                    act.copy(cur_tiles[0][:, 4 * q:4 * q + 4, :], psa[:])
                    act.copy(cur_tiles[1][:, 4 * q:4 * q + 4, :], psb[:])

            def emit_gemm_rec(ch, rec_steps):
                """GEMM of chunk ch with rec steps (list of (chunk, step))
                interleaved two per 4-tile bundle."""
                new_chunk_tiles(ch)
                ri = 0
                for q in range(NT // 4):
                    emit_bundle(ch, q)
                    for _ in range(2):
                        if ri < len(rec_steps):
                            rc, rt = rec_steps[ri]
                            emit_rec_step(rc, rt)
                            ri += 1
                while ri < len(rec_steps):
                    rc, rt = rec_steps[ri]
                    emit_rec_step(rc, rt)
                    ri += 1

            def emit_gemm_blocks(ch, rec_steps, soma_pool=False):
                """GEMM of chunk ch in 2-step column blocks, rec steps
                interleaved two per block (chain rides inside the GEMM)."""
                new_chunk_tiles(ch)
                ri = 0
                for b in range(NTB):
                    cols = slice(b * TB * BS, (b + 1) * TB * BS)
                    for q in range(NT // 4):
                        ps = tpsum.tile([128, 4, TB * BS], F32, tag="tps")
                        for j in range(4):
                            emit_mm_group(ch, 4 * q + j, ps[:, j, :], cols)
                        act.copy(
                            cur_tiles[ch][:, 4 * q:4 * q + 4, cols], ps[:])
                    for _ in range(2):
                        if ri < len(rec_steps):
                            rc, rt = rec_steps[ri]
                            emit_rec_step(rc, rt, soma_pool)
                            ri += 1
                while ri < len(rec_steps):
                    rc, rt = rec_steps[ri]
                    emit_rec_step(rc, rt, soma_pool)
                    ri += 1

            def emit_outgemm_half(ch, half):
                """out-layer GEMM on half of chunk ch's soma sign-spikes.
                cot = 0.5*w2^T @ zb_s + c0  (bias applied in the evac)."""
                zb = zb_tiles[ch]
                cols = slice(half * NCOL // 2, (half + 1) * NCOL // 2)
                if half == 0:
                    co_tiles[ch] = copool.tile([40, NCOL], F32, tag="cot",
                                               name=f"cot{ch}")
                cot = co_tiles[ch]
                ops = opsum.tile([40, NCOL // 2], F32, tag="ops")
                for g in range(ST):
                    nc.tensor.matmul(
                        ops[:], w2s[:, g, :], zb[:, g, cols],
                        start=(g == 0), stop=(g == ST - 1))
                act.copy(cot[:, cols], ops[:])

            def emit_outgemm(ch):
                emit_outgemm_half(ch, 0)
                emit_outgemm_half(ch, 1)

            def emit_outlayer(ch):
                """out-dendrite LIF for chunk ch on Pool ([40, BS]);
                ships the chunk's zq slab to DRAM when done."""
                cot = co_tiles[ch]
                zqt = zqpool.tile([40, CH, BS], I8, tag="zq",
                                  name=f"zq{ch}")
                for tl in range(CH):
                    col = slice(tl * BS, (tl + 1) * BS)
                    # V1: q = 0.9 q + ido(old)   (Pool has no fused STT)
                    gp.tensor_scalar(qo[:], qo[:], 0.9, None, OP.mult)
                    gp.tensor_tensor(qo[:], qo[:], ido[:], OP.add)
                    # O: ido = 0.8 ido + cur_o(t)
                    gp.tensor_scalar(ido[:], ido[:], 0.8, None, OP.mult)
                    gp.tensor_tensor(ido[:], ido[:], cot[:, col], OP.add)
                    # Z: zq(t) = (q > 10)
                    gp.tensor_scalar(
                        zqt[:, tl, :], qo[:], 10.0, None, OP.is_gt)
                    # QR: q = (q<=10)*q
                    qm = zspool.tile([40, BS], F16, tag="qm")
                    gp.tensor_scalar(qm[:], qo[:], 10.0, None, OP.is_le)
                    gp.tensor_tensor(qo[:], qo[:], qm[:], OP.mult)
                nc.sync.dma_start(
                    outd[:, ch * CH:(ch + 1) * CH].rearrange(
                        "p t b -> p (t b)"),
                    zqt[:].rearrange("p t b -> p (t b)"))

            # ---- main pipeline ----
            emit_gemm01()
            # chunk 2 carries rec(0)+rec(1); later chunks carry rec(ch-1);
            # chunks 5..7 are column-blocked so the recurrence chain rides
            # inside their GEMM windows. Out stages run at lag 3.
            emit_gemm_rec(2, [(0, t) for t in range(CH)]
                          + [(1, t) for t in range(CH)])
            emit_outgemm(0)
            emit_outlayer(0)
            emit_gemm_rec(3, [(2, t) for t in range(CH)])
            emit_outgemm(1)
            emit_outlayer(1)
            emit_gemm_rec(4, [(3, t) for t in range(CH)])
            emit_outgemm(2)
            emit_outlayer(2)
            emit_gemm_blocks(5, [(4, t) for t in range(CH)])
            emit_outgemm(3)
            emit_outlayer(3)
            emit_gemm_blocks(6, [(5, t) for t in range(CH)])
            emit_outgemm(4)
            emit_outlayer(4)
            emit_outgemm(5)
            emit_outlayer(5)
            emit_gemm_blocks(7, [(6, t) for t in range(CH)],
                             soma_pool=True)
            emit_outgemm(NCH - 2)
            emit_outlayer(NCH - 2)

            # ---- chunk-7 recurrence + per-2-step out drain ----
            chD = NCH - 1
            cotD = copool.tile([40, NCOL], F32, tag="cot", name="cotD")
            co_tiles[chD] = cotD
            zqD = zqpool.tile([40, CH, BS], I8, tag="zq", name="zqD")
            for tl in range(CH):
                emit_rec_step(chD, tl, soma_pool=True)
                if tl % TB == TB - 1:
                    cols = slice((tl - 1) * BS, (tl + 1) * BS)
                    ops = dpsum.tile([40, TB * BS], F32, tag="opsD")
                    zb = zb_tiles[chD]
                    for g in range(ST):
                        nc.tensor.matmul(
                            ops[:], w2s[:, g, :], zb[:, g, cols],
                            start=(g == 0), stop=(g == ST - 1))
                    act.copy(cotD[:, cols], ops[:])
                    for tq in (tl - 1, tl):
                        col = slice(tq * BS, (tq + 1) * BS)
                        gp.tensor_scalar(qo[:], qo[:], 0.9, None, OP.mult)
                        gp.tensor_tensor(qo[:], qo[:], ido[:], OP.add)
                        gp.tensor_scalar(ido[:], ido[:], 0.8, None, OP.mult)
                        gp.tensor_tensor(ido[:], ido[:], cotD[:, col], OP.add)
                        gp.tensor_scalar(
                            zqD[:, tq, :], qo[:], 10.0, None, OP.is_gt)
                        qm = zspool.tile([40, BS], F16, tag="qm")
                        gp.tensor_scalar(qm[:], qo[:], 10.0, None, OP.is_le)
                        gp.tensor_tensor(qo[:], qo[:], qm[:], OP.mult)

            nc.sync.dma_start(
                outd[:, (NCH - 1) * CH:T].rearrange("p t b -> p (t b)"),
                zqD[:].rearrange("p t b -> p (t b)"))

    nc.finalize()
    return nc


def prep_inputs(x, w_hidden, w_out):
    """Host-side shard + repack. Returns per-core input maps."""
    x = np.ascontiguousarray(x, dtype=np.float32)
    # synaptic pre-filter, shifted one step (slot t holds x~(t-1))
    xf = np.zeros((T + 1, B, FS2), np.float32)
    acc = np.zeros(x.shape[1:], np.float32)
    for t in range(T - 1):
        acc = acc * np.float32(0.8) + x[t]
        xf[t + 1] = acc
    xh = xf[:T].astype(np.float16)
    whh = np.asarray(w_hidden, np.float32).astype(np.float16)
    woh = np.asarray(w_out, np.float32).astype(np.float16)
    # w_hidden [HC, SPL1, H1] -> [HC, MT, 128p, KT, 128q]
    whp = np.ascontiguousarray(
        whh.reshape(HC, KT, 128, MT, 128).transpose(0, 3, 2, 1, 4))
    # w_out [OC, SPL2, OUT] -> dense block-diagonal, halved for the
    # sign-spike encoding: cur_o = (w2/2)^T @ zb_s + c0, c0 = rowsum(w2/2)
    w2 = np.zeros((ST, 128, 40), np.float16)
    for g in range(ST):
        for i in range(128):
            f = g * 128 + i
            c = f // SPL2
            w2[g, i, c * OUT:(c + 1) * OUT] = woh[c, f % SPL2, :]
    in_maps = []
    for i in range(N_CORES):
        xs_ = xh[:, i * BS:(i + 1) * BS, :]              # [T, BS, FS2]
        xt = np.ascontiguousarray(
            xs_.reshape(ROWS, FS2).T.reshape(GT, 128, ROWS).transpose(1, 0, 2))
        # [128, GT, ROWS] -> chunk-major [128, NCH, GT, NCOL]
        xt = np.ascontiguousarray(
            xt.reshape(128, GT, NCH, NCOL).transpose(0, 2, 1, 3))
        in_maps.append({"xT": xt, "wh": whp, "w2": w2})
    return in_maps


def _readout_kernel():
    """Kcomb[t, u]: vso(t) = sum_u Kcomb[t,u] * zq-count(u)."""
    Kc = np.zeros((T, T), np.float64)
    for t in range(T):
        for u in range(t):          # iso(s) for s in [u, t-1]
            s = np.arange(u, t)
            Kc[t, u] = 0.1 * np.sum(0.9 ** (t - 1 - s) * 0.8 ** (s - u))
    return Kc.astype(np.float32)


_KCOMB = _readout_kernel()
_NC_CACHE = {}


def get_nc(repeat=1):
    if repeat not in _NC_CACHE:
        _NC_CACHE[repeat] = build_nc(repeat)
    return _NC_CACHE[repeat]


def run(inputs, trace=False, repeat=1, **kw):
    """Returns (full_output [T,B,10], BassKernelResults)."""
    nc = get_nc(repeat)
    in_maps = prep_inputs(inputs["x"], inputs["w_hidden"], inputs["w_out"])
    res = run_bass_kernel_spmd(nc, in_maps, list(range(N_CORES)),
                               trace=trace, **kw)
    out = np.empty((T, B, OUT), dtype=np.float32)
    for i in range(N_CORES):
        zq = np.asarray(res.results[i]["outd"]).astype(np.float32)
        zq = zq.reshape(OC, OUT, T, BS)
        v = np.einsum('tu,oub->tbo', _KCOMB, zq.sum(0), optimize=True)
        out[:, i * BS:(i + 1) * BS, :] = v
    return out, res


def kernel(x, w_hidden, w_out):
    out, _ = run({"x": x, "w_hidden": w_hidden, "w_out": w_out})
    return out
